# revision 1
# baseline (speedup 1.0000x reference)
"""BitFeedForward (ternary-weight SwiGLU-ish FFN) on 8 Trainium2 NeuronCores.

Strategy: data-parallel over tokens (8192 tokens -> 1024/core). Weights are
ternarized on host (exact {-1,0,+1} in bf16); activations are int8-value
quantized on device (integers are exact in bf16), so every matmul runs on the
PE at full bf16 rate and the integer accumulations in fp32 PSUM are exact.

Single-pass structure per core (T=1024 tokens, D=2048, H=8192):
  A: x -> rmsnorm stats -> int8 q1/q2 (token-major) -> DRAM -> XBAR-transposed
     feature-major q1T/q2T in SBUF.
  B: mm1+mm2 streaming w1/w2 once (N=256 psum tiles), fused
     silu(c1*u)*(c2*v) -> h, g3*h staged to DRAM, running sum(h^2)/max|g3 h|.
  C: finalize per-token scales for the output quant.
  D: re-quantize g3*h -> q3 (token-major) -> DRAM; mm3 over 2 token-groups x
     2 D-halves with q3T chunks XBAR-transposed from DRAM per hc; w3 is
     streamed twice in halves; per-token c3 scaling on evacuation.
"""

import sys

sys.path.insert(0, "/opt/trn_rl_repo")

import numpy as np
import ml_dtypes

import concourse.bass as bass
from concourse import bacc, mybir
from concourse.bass_utils import run_bass_kernel_spmd
from concourse.tile import TileContext

# problem dims
B, S, D, H = 4, 2048, 2048, 8192
NTOK = B * S            # 8192 tokens
NCORES = 8
T_CORE = NTOK // NCORES  # 1024 tokens per core

EPS = 1e-8
C_RINT = float(1.5 * 2.0**23)   # (y + C) - C == rint(y) for |y| < 2^22
ATANH_HALF = float(np.arctanh(np.float64(0.5)))

F32 = mybir.dt.float32
BF16 = mybir.dt.bfloat16

# device loop constants
TT = 8                   # 128-token tiles
HBW = 256                # H columns per mm12 block
HB = H // HBW            # 32
DC = D // 128            # 16 contraction chunks for mm1/2
HC = H // 128            # 64 contraction chunks for mm3
PIECE = 8                # h blocks per pass-2 piece (8*256 = 2048 H)
NPIECE = HB // PIECE     # 4
TG = 2                   # token groups for mm3 (4 tiles each)
DH = 2                   # D halves for mm3


def _build_program():
    nc = bacc.Bacc("TRN2", target_bir_lowering=False, debug=False)

    x_d = nc.dram_tensor("x", [T_CORE, D], F32, kind="ExternalInput")
    w1_d = nc.dram_tensor("w1q", [D, H], BF16, kind="ExternalInput")
    w2_d = nc.dram_tensor("w2q", [D, H], BF16, kind="ExternalInput")
    w3_d = nc.dram_tensor("w3q", [H, D], BF16, kind="ExternalInput")
    g1_d = nc.dram_tensor("g1", [1, D], BF16, kind="ExternalInput")
    g2_d = nc.dram_tensor("g2", [1, D], BF16, kind="ExternalInput")
    g3_d = nc.dram_tensor("g3", [1, H], BF16, kind="ExternalInput")
    kc_d = nc.dram_tensor("kconst", [1, 3], F32, kind="ExternalInput")
    out_d = nc.dram_tensor("out", [T_CORE, D], F32, kind="ExternalOutput")
    # g3*h staged per (toktile, hblock): [tt, hb, p, c]
    gh_d = nc.dram_tensor("gh_scratch", [TT, HB, 128, HBW], F32)
    gh_r = gh_d.rearrange("t hb p c -> t p hb c")
    # token-major quantized activations staged for XBAR transpose loads
    q1_d = nc.dram_tensor("q1_scratch", [T_CORE, D], BF16)
    q2_d = nc.dram_tensor("q2_scratch", [T_CORE, D], BF16)
    q3_d = nc.dram_tensor("q3_scratch", [T_CORE, H], BF16)

    w1_r = w1_d.rearrange("(dc p) h -> p dc h", p=128)
    w2_r = w2_d.rearrange("(dc p) h -> p dc h", p=128)

    with TileContext(nc) as tc, bass.ExitStack() as ctx:
        ec = ctx.enter_context
        singles = ec(tc.tile_pool(name="singles", bufs=1))
        wpool = ec(tc.tile_pool(name="wpool", bufs=2))
        xpool = ec(tc.tile_pool(name="xpool", bufs=2))
        scr = ec(tc.tile_pool(name="scr", bufs=2))
        qb = ec(tc.tile_pool(name="qb", bufs=2))
        hpool = ec(tc.tile_pool(name="hpool", bufs=10))
        stats = ec(tc.tile_pool(name="stats", bufs=1))
        parts = ec(tc.tile_pool(name="parts", bufs=4))
        hload = ec(tc.tile_pool(name="hload", bufs=2))
        q3pool = ec(tc.tile_pool(name="q3pool", bufs=2))
        q3tc = ec(tc.tile_pool(name="q3tc", bufs=3))
        w3pool = ec(tc.tile_pool(name="w3pool", bufs=2))
        outp = ec(tc.tile_pool(name="outp", bufs=2))
        psum = ec(tc.tile_pool(name="psum", bufs=8, space="PSUM"))

        # ---- constants ----
        epst = singles.tile([128, 1], F32, tag="eps")
        nc.vector.memset(epst, EPS)
        g1rep = singles.tile([128, D], BF16, tag="g1rep")
        nc.sync.dma_start(out=g1rep, in_=g1_d[:, :].to_broadcast([128, D]))
        g2rep = singles.tile([128, D], BF16, tag="g2rep")
        nc.sync.dma_start(out=g2rep, in_=g2_d[:, :].to_broadcast([128, D]))
        g3rep = singles.tile([128, H], BF16, tag="g3rep")
        nc.sync.dma_start(out=g3rep, in_=g3_d[:, :].to_broadcast([128, H]))
        karep = singles.tile([128, 3], F32, tag="karep")
        nc.sync.dma_start(out=karep, in_=kc_d[:, :].to_broadcast([128, 3]))

        # persistent feature-major activations
        q1T = singles.tile([128, DC, T_CORE], BF16, tag="q1T")
        q2T = singles.tile([128, DC, T_CORE], BF16, tag="q2T")

        # per-token-tile stats [128, TT]
        c1_t = stats.tile([128, TT], F32, tag="c1")
        c2_t = stats.tile([128, TT], F32, tag="c2")
        c3_t = stats.tile([128, TT], F32, tag="c3")
        rho3_t = stats.tile([128, TT], F32, tag="rho3")
        S3_t = stats.tile([128, TT], F32, tag="S3")
        M3_t = stats.tile([128, TT], F32, tag="M3")
        r_t = stats.tile([128, TT], F32, tag="r1")

        def tok_scalars(dst_c, dst_rho, M_ap, r_ap, kcol):
            """denom = max(M*r, 1e-4); dst_c = denom * karep[:,kcol];
            dst_rho = 127 * r / denom."""
            den = parts.tile([128, 1], F32, tag="den")
            nc.vector.tensor_tensor(out=den, in0=M_ap, in1=r_ap,
                                    op=mybir.AluOpType.mult)
            nc.vector.tensor_scalar_max(out=den, in0=den, scalar1=1e-4)
            nc.vector.tensor_scalar(out=dst_c, in0=den,
                                    scalar1=karep[:, kcol:kcol + 1], scalar2=None,
                                    op0=mybir.AluOpType.mult)
            iden = parts.tile([128, 1], F32, tag="iden")
            nc.vector.reciprocal(out=iden, in_=den)
            nc.vector.tensor_tensor(out=iden, in0=iden, in1=r_ap,
                                    op=mybir.AluOpType.mult)
            nc.vector.tensor_scalar(out=dst_rho, in0=iden, scalar1=127.0,
                                    scalar2=None, op0=mybir.AluOpType.mult)

        # ======== phase A: x prep -> q1/q2 -> feature-major q1T/q2T
        for tt in range(TT):
            tok0 = tt * 128
            x_t = xpool.tile([128, D], F32, tag="x")
            nc.sync.dma_start(out=x_t, in_=x_d[tok0:tok0 + 128, :])
            sink = scr.tile([128, D], F32, tag="scr")
            ssq = parts.tile([128, 1], F32, tag="ssq")
            nc.scalar.activation(out=sink, in_=x_t,
                                 func=mybir.ActivationFunctionType.Square,
                                 accum_out=ssq)
            # r = 1/sqrt(ssq/D + eps)
            nc.scalar.activation(out=r_t[:, tt:tt + 1], in_=ssq,
                                 func=mybir.ActivationFunctionType.Sqrt,
                                 bias=epst, scale=1.0 / D)
            nc.vector.reciprocal(out=r_t[:, tt:tt + 1], in_=r_t[:, tt:tt + 1])

            for (grep, q_dram, c_dst, kcol) in (
                (g1rep, q1_d, c1_t, 0),
                (g2rep, q2_d, c2_t, 1),
            ):
                gx = scr.tile([128, D], F32, tag="scr")
                nc.vector.tensor_tensor(out=gx, in0=x_t, in1=grep,
                                        op=mybir.AluOpType.mult)
                M = parts.tile([128, 1], F32, tag="M")
                nc.vector.tensor_reduce(out=M, in_=gx,
                                        axis=mybir.AxisListType.X,
                                        op=mybir.AluOpType.max,
                                        apply_absolute_value=True)
                rho = parts.tile([128, 1], F32, tag="rho")
                tok_scalars(c_dst[:, tt:tt + 1], rho, M, r_t[:, tt:tt + 1], kcol)
                # q = rint(gx * rho) via magic constant, cast to bf16
                nc.vector.tensor_scalar(out=gx, in0=gx, scalar1=rho,
                                        scalar2=C_RINT,
                                        op0=mybir.AluOpType.mult,
                                        op1=mybir.AluOpType.add)
                qt = qb.tile([128, D], BF16, tag="qb")
                nc.vector.tensor_scalar(out=qt, in0=gx, scalar1=C_RINT,
                                        scalar2=None,
                                        op0=mybir.AluOpType.subtract)
                nc.sync.dma_start(out=q_dram[tok0:tok0 + 128, :], in_=qt)
        # feature-major loads via XBAR transpose (d = dc*128 + p layout)
        nc.scalar.dma_start_transpose(q1T, q1_d[:, :])
        nc.scalar.dma_start_transpose(q2T, q2_d[:, :])

        # ======== phase B: mm1/mm2 + h + stats, streaming w1/w2 once
        for hb in range(HB):
            w1b = wpool.tile([128, DC, HBW], BF16, tag="w1b")
            nc.sync.dma_start(out=w1b, in_=w1_r[:, :, hb * HBW:(hb + 1) * HBW])
            w2b = wpool.tile([128, DC, HBW], BF16, tag="w2b")
            nc.sync.dma_start(out=w2b, in_=w2_r[:, :, hb * HBW:(hb + 1) * HBW])
            for tt in range(TT):
                pu = psum.tile([128, HBW], F32, tag="ps")
                for dc in range(DC):
                    nc.tensor.matmul(pu, lhsT=q1T[:, dc, tt * 128:(tt + 1) * 128],
                                     rhs=w1b[:, dc, :],
                                     start=(dc == 0), stop=(dc == DC - 1))
                pv = psum.tile([128, HBW], F32, tag="ps")
                for dc in range(DC):
                    nc.tensor.matmul(pv, lhsT=q2T[:, dc, tt * 128:(tt + 1) * 128],
                                     rhs=w2b[:, dc, :],
                                     start=(dc == 0), stop=(dc == DC - 1))
                sg = hpool.tile([128, HBW], F32, tag="h")
                nc.scalar.activation(out=sg, in_=pu,
                                     func=mybir.ActivationFunctionType.Sigmoid,
                                     scale=c1_t[:, tt:tt + 1])
                ur = hpool.tile([128, HBW], F32, tag="h")
                nc.scalar.mul(out=ur, in_=pu, mul=c1_t[:, tt:tt + 1])
                swish = hpool.tile([128, HBW], F32, tag="h")
                nc.vector.tensor_tensor(out=swish, in0=sg, in1=ur,
                                        op=mybir.AluOpType.mult)
                vre = hpool.tile([128, HBW], F32, tag="h")
                nc.scalar.mul(out=vre, in_=pv, mul=c2_t[:, tt:tt + 1])
                ht = hpool.tile([128, HBW], F32, tag="h")
                nc.vector.tensor_tensor(out=ht, in0=swish, in1=vre,
                                        op=mybir.AluOpType.mult)
                # sum(h^2) accumulate
                sinkh = hpool.tile([128, HBW], F32, tag="h")
                sp = parts.tile([128, 1], F32, tag="sp")
                nc.scalar.activation(out=sinkh, in_=ht,
                                     func=mybir.ActivationFunctionType.Square,
                                     accum_out=sp)
                if hb == 0:
                    nc.vector.tensor_copy(out=S3_t[:, tt:tt + 1], in_=sp)
                else:
                    nc.vector.tensor_tensor(out=S3_t[:, tt:tt + 1],
                                            in0=S3_t[:, tt:tt + 1], in1=sp,
                                            op=mybir.AluOpType.add)
                # gh = g3*h (stored to DRAM); max|gh| accumulate
                gh = hpool.tile([128, HBW], F32, tag="h")
                nc.vector.tensor_tensor(out=gh, in0=ht,
                                        in1=g3rep[:, hb * HBW:(hb + 1) * HBW],
                                        op=mybir.AluOpType.mult)
                mp = parts.tile([128, 1], F32, tag="mp")
                nc.vector.tensor_reduce(out=mp, in_=gh,
                                        axis=mybir.AxisListType.X,
                                        op=mybir.AluOpType.max,
                                        apply_absolute_value=True)
                if hb == 0:
                    nc.vector.tensor_copy(out=M3_t[:, tt:tt + 1], in_=mp)
                else:
                    nc.vector.tensor_tensor(out=M3_t[:, tt:tt + 1],
                                            in0=M3_t[:, tt:tt + 1], in1=mp,
                                            op=mybir.AluOpType.max)
                nc.sync.dma_start(out=gh_d[tt, hb], in_=gh)

        # ======== phase C: finalize h stats
        for tt in range(TT):
            r3 = parts.tile([128, 1], F32, tag="r3")
            nc.scalar.activation(out=r3, in_=S3_t[:, tt:tt + 1],
                                 func=mybir.ActivationFunctionType.Sqrt,
                                 bias=epst, scale=1.0 / H)
            nc.vector.reciprocal(out=r3, in_=r3)
            tok_scalars(c3_t[:, tt:tt + 1], rho3_t[:, tt:tt + 1],
                        M3_t[:, tt:tt + 1], r3, 2)

        # ======== phase D: pass 2 (q3 quantize) + mm3
        for tg in range(TG):
            # quantize this token-group's gh -> q3 (token-major) in DRAM
            for ttl in range(TT // TG):
                tt = tg * (TT // TG) + ttl
                tok0 = tt * 128
                for pc in range(NPIECE):
                    hl = hload.tile([128, PIECE, HBW], F32, tag="hl")
                    nc.sync.dma_start(
                        out=hl,
                        in_=gh_r[tt, :, pc * PIECE:(pc + 1) * PIECE, :])
                    nc.vector.tensor_scalar(out=hl, in0=hl,
                                            scalar1=rho3_t[:, tt:tt + 1],
                                            scalar2=C_RINT,
                                            op0=mybir.AluOpType.mult,
                                            op1=mybir.AluOpType.add)
                    q3p = q3pool.tile([128, PIECE * HBW], BF16, tag="q3p")
                    nc.vector.tensor_scalar(
                        out=q3p,
                        in0=hl.rearrange("p a c -> p (a c)"),
                        scalar1=C_RINT, scalar2=None,
                        op0=mybir.AluOpType.subtract)
                    nc.sync.dma_start(
                        out=q3_d[tok0:tok0 + 128,
                                 pc * PIECE * HBW:(pc + 1) * PIECE * HBW],
                        in_=q3p)
            gtok0 = tg * 512
            for dh in range(DH):
                pos = [psum.tile([128, 512], F32, tag="ps",
                                 name=f"po{tg}_{dh}_{i}") for i in range(8)]
                for hc in range(HC):
                    # q3T chunk [128 H, 512 tok] via XBAR transpose from DRAM
                    q3c = q3tc.tile([128, 512], BF16, tag="q3c")
                    nc.scalar.dma_start_transpose(
                        q3c, q3_d[gtok0:gtok0 + 512, hc * 128:(hc + 1) * 128])
                    w3b = w3pool.tile([128, 1024], BF16, tag="w3b")
                    nc.sync.dma_start(
                        out=w3b,
                        in_=w3_d[hc * 128:(hc + 1) * 128,
                                 dh * 1024:(dh + 1) * 1024])
                    for ttl in range(4):
                        for dc3 in range(2):
                            nc.tensor.matmul(
                                pos[ttl * 2 + dc3],
                                lhsT=q3c[:, ttl * 128:(ttl + 1) * 128],
                                rhs=w3b[:, dc3 * 512:(dc3 + 1) * 512],
                                start=(hc == 0), stop=(hc == HC - 1),
                                skip_group_check=True)
                for ttl in range(4):
                    tt = tg * 4 + ttl
                    tok0 = tt * 128
                    for dc3 in range(2):
                        ob = outp.tile([128, 512], F32, tag="ob")
                        nc.scalar.mul(out=ob, in_=pos[ttl * 2 + dc3],
                                      mul=c3_t[:, tt:tt + 1])
                        dcol = dh * 1024 + dc3 * 512
                        nc.sync.dma_start(
                            out=out_d[tok0:tok0 + 128, dcol:dcol + 512],
                            in_=ob)

    nc.compile()
    return nc


_NC_CACHE = []


def _get_program():
    if not _NC_CACHE:
        _NC_CACHE.append(_build_program())
    return _NC_CACHE[0]


def _ternary_T(w):
    """Host ternarization matching round(tanh(w/(mean|w|+eps))) in value.
    Uses CPU-jax to replicate the reference's fp32 tanh bit-for-bit.
    Returns (transposed ternary bf16 array, arctanh(s) as float32)."""
    w32 = np.asarray(w, dtype=np.float32)
    try:
        import jax
        import jax.numpy as jnp
        cpu = jax.devices("cpu")[0]
        with jax.default_device(cpu):
            s = jnp.mean(jnp.abs(jnp.asarray(w32)))
            t = np.asarray(jnp.round(jnp.tanh(w32 / (s + np.float32(EPS)))))
            a = np.float32(jnp.arctanh(s))
    except Exception:
        s32 = np.float32(np.mean(np.abs(w32), dtype=np.float64))
        denom = np.float32(s32 + np.float32(EPS))
        thresh = np.float32(ATANH_HALF) * denom
        t = np.sign(w32) * (np.abs(w32) > thresh)
        a = np.float32(np.arctanh(np.float64(s32)))
    return np.ascontiguousarray(t.T).astype(ml_dtypes.bfloat16), a


def kernel(x, w1, g1, w2, g2, w3, g3):
    nc = _get_program()

    x32 = np.asarray(x, np.float32).reshape(NTOK, D)
    w1q, a1 = _ternary_T(w1)            # [D, H]
    w2q, a2 = _ternary_T(w2)            # [D, H]
    w3q, a3 = _ternary_T(w3)            # [H, D] (w3 is [D, H])
    g1b = np.asarray(g1, np.float32).reshape(1, D).astype(ml_dtypes.bfloat16)
    g2b = np.asarray(g2, np.float32).reshape(1, D).astype(ml_dtypes.bfloat16)
    g3b = np.asarray(g3, np.float32).reshape(1, H).astype(ml_dtypes.bfloat16)
    kconst = np.array([[a1 / 127.0, a2 / 127.0, a3 / 127.0]], np.float32)

    in_maps = []
    for c in range(NCORES):
        in_maps.append({
            "x": np.ascontiguousarray(x32[c * T_CORE:(c + 1) * T_CORE]),
            "w1q": w1q, "w2q": w2q, "w3q": w3q,
            "g1": g1b, "g2": g2b, "g3": g3b,
            "kconst": kconst,
        })
    res = run_bass_kernel_spmd(nc, in_maps, list(range(NCORES)))
    out = np.concatenate([res.results[c]["out"] for c in range(NCORES)], axis=0)
    return out.reshape(B, S, D)



# revision 6
# speedup vs baseline: 1.7258x; 1.7258x over previous
"""BitFeedForward (ternary-weight SwiGLU-ish FFN) on 8 Trainium2 NeuronCores.

Strategy: data-parallel over tokens (8192 tokens -> 1024/core). Weights are
ternarized on host (exact {-1,0,+1} in bf16); activations are int8-value
quantized on device (integers are exact in bf16), so every matmul runs on the
PE at full bf16 rate and the integer accumulations in fp32 PSUM are exact.

v2 layout: phase B computes u/v FEATURE-major (psum = [128 H, 512 tok]) by
using the weight chunk as the stationary operand and the feature-major
quantized activations as the moving operand. h/gh then come out feature-major,
so the mm3 contraction over H needs NO transposes at all: gh is staged to DRAM
[H, T] f32, reloaded chunk-wise, quantized in place (rho3 broadcast along
tokens), and fed straight to the PE as lhsT. w3 is streamed exactly once.

  A: x -> rmsnorm stats -> int8 q1/q2 (token-major) -> DRAM -> XBAR-transposed
     feature-major q1T/q2T in SBUF; c1/c2 transposed to token-broadcast rows.
  B: mm1+mm2 feature-major (N=512 moving tokens), fused silu(c1*u)*(c2*v),
     g3*h staged to DRAM [H,T], running elementwise sum(h^2)/max|g3 h| in
     [128, T] accumulators.
  C: PE-transpose the accumulators -> per-token S3/M3 -> c3/rho3; rho3
     broadcast along tokens via a DRAM bounce.
  D: reload gh chunks [128 H, 1024 tok], quantize -> q3T chunks (bf16),
     mm3 over 4 D-quarters x 64 H-chunks into 8 token-tile psum banks.
"""

import sys

sys.path.insert(0, "/opt/trn_rl_repo")

import numpy as np
import ml_dtypes

import concourse.bass as bass
from concourse import bacc, mybir
from concourse.bass_utils import run_bass_kernel_spmd
from concourse.tile import TileContext
from concourse.masks import make_identity

# problem dims
B, S, D, H = 4, 2048, 2048, 8192
NTOK = B * S            # 8192 tokens
NCORES = 8
T_CORE = NTOK // NCORES  # 1024 tokens per core

EPS = 1e-8
C_RINT = float(1.5 * 2.0**23)   # (y + C) - C == rint(y) for |y| < 2^22
ATANH_HALF = float(np.arctanh(np.float64(0.5)))

F32 = mybir.dt.float32
BF16 = mybir.dt.bfloat16

# device loop constants
TT = 8                   # 128-token tiles
TH = 2                   # 512-token halves (phase B moving dim)
HBW = 256                # H columns per w1/w2 stream block
HB = H // HBW            # 32
DC = D // 128            # 16 contraction chunks for mm1/2
HC = H // 128            # 64 contraction chunks for mm3
DQ = 4                   # D quarters for mm3 (512 cols each)


def _build_program():
    nc = bacc.Bacc("TRN2", target_bir_lowering=False, debug=False)

    x_d = nc.dram_tensor("x", [T_CORE, D], F32, kind="ExternalInput")
    w1_d = nc.dram_tensor("w1q", [D, H], BF16, kind="ExternalInput")
    w2_d = nc.dram_tensor("w2q", [D, H], BF16, kind="ExternalInput")
    w3_d = nc.dram_tensor("w3q", [H, D], BF16, kind="ExternalInput")
    g1_d = nc.dram_tensor("g1", [1, D], F32, kind="ExternalInput")
    g2_d = nc.dram_tensor("g2", [1, D], F32, kind="ExternalInput")
    g3c_d = nc.dram_tensor("g3c", [128, HC], F32, kind="ExternalInput")
    kc_d = nc.dram_tensor("kconst", [1, 3], F32, kind="ExternalInput")
    out_d = nc.dram_tensor("out", [T_CORE, D], F32, kind="ExternalOutput")
    # g3*h staged feature-major: [H, T] f32
    gh_d = nc.dram_tensor("gh_scratch", [H, T_CORE], F32)
    # token-major quantized activations staged for XBAR transpose loads
    q1_d = nc.dram_tensor("q1_scratch", [T_CORE, D], BF16)
    q2_d = nc.dram_tensor("q2_scratch", [T_CORE, D], BF16)
    # stat bounce buffers for token-broadcast rows
    cstat_d = nc.dram_tensor("cstat", [1, 16 * 128], F32)
    rstat_d = nc.dram_tensor("rstat", [1, TT * 128], F32)

    w1_r = w1_d.rearrange("(dc p) h -> p dc h", p=128)
    w2_r = w2_d.rearrange("(dc p) h -> p dc h", p=128)

    with TileContext(nc) as tc, bass.ExitStack() as ctx:
        ec = ctx.enter_context
        singles = ec(tc.tile_pool(name="singles", bufs=1))
        parts = ec(tc.tile_pool(name="parts", bufs=4))

        # ---- persistent constants / stats ----
        epst = singles.tile([128, 1], F32, tag="eps")
        nc.vector.memset(epst, EPS)
        ident = singles.tile([128, 128], F32, tag="ident")
        make_identity(nc, ident)
        karep = singles.tile([128, 3], F32, tag="karep")
        nc.sync.dma_start(out=karep, in_=kc_d[:, :].to_broadcast([128, 3]))
        g3c_t = singles.tile([128, HC], F32, tag="g3c")
        nc.sync.dma_start(out=g3c_t, in_=g3c_d[:, :])

        # token-major per-token stats (partition = token within tile tt)
        c3_t = singles.tile([128, TT], F32, tag="c3")
        rho3_t = singles.tile([128, TT], F32, tag="rho3")
        S3_t = singles.tile([128, TT], F32, tag="S3")
        M3_t = singles.tile([128, TT], F32, tag="M3")
        r_t = singles.tile([128, TT], F32, tag="r1")
        # token-broadcast rho3 (used by phase D quantize)
        rho3rep = singles.tile([128, T_CORE], F32, tag="rho3rep")

        def tok_scalars(dst_c, dst_rho, M_ap, r_ap, kcol):
            """denom = max(M*r, 1e-4); dst_c = denom * karep[:,kcol];
            dst_rho = 127 * r / denom."""
            den = parts.tile([128, 1], F32, tag="den")
            nc.vector.tensor_tensor(out=den, in0=M_ap, in1=r_ap,
                                    op=mybir.AluOpType.mult)
            nc.vector.tensor_scalar_max(out=den, in0=den, scalar1=1e-4)
            nc.vector.tensor_scalar(out=dst_c, in0=den,
                                    scalar1=karep[:, kcol:kcol + 1], scalar2=None,
                                    op0=mybir.AluOpType.mult)
            iden = parts.tile([128, 1], F32, tag="iden")
            nc.vector.reciprocal(out=iden, in_=den)
            nc.vector.tensor_tensor(out=iden, in0=iden, in1=r_ap,
                                    op=mybir.AluOpType.mult)
            nc.vector.tensor_scalar(out=dst_rho, in0=iden, scalar1=127.0,
                                    scalar2=None, op0=mybir.AluOpType.mult)

        with tc.tile_pool(name="bscope", bufs=1) as bsc, \
             tc.tile_pool(name="xpool", bufs=2) as xpool, \
             tc.tile_pool(name="scr", bufs=3) as scr, \
             tc.tile_pool(name="qb", bufs=2) as qb, \
             tc.tile_pool(name="wpool", bufs=2) as wpool, \
             tc.tile_pool(name="hpool", bufs=10) as hpool, \
             tc.tile_pool(name="psB", bufs=6, space="PSUM") as psB:

            g1rep = bsc.tile([128, D], F32, tag="g1rep")
            nc.sync.dma_start(out=g1rep, in_=g1_d[:, :].to_broadcast([128, D]))
            g2rep = bsc.tile([128, D], F32, tag="g2rep")
            nc.sync.dma_start(out=g2rep, in_=g2_d[:, :].to_broadcast([128, D]))

            # persistent feature-major activations
            q1T = bsc.tile([128, DC, T_CORE], BF16, tag="q1T")
            q2T = bsc.tile([128, DC, T_CORE], BF16, tag="q2T")
            # c1/c2 stacked token-major for one transpose -> broadcast
            cstack = bsc.tile([128, 16], F32, tag="cstack")
            # token-broadcast c1/c2 rows
            c1rep = bsc.tile([128, T_CORE], F32, tag="c1rep")
            c2rep = bsc.tile([128, T_CORE], F32, tag="c2rep")
            # elementwise stat accumulators [128, T]
            S3run = bsc.tile([128, T_CORE], F32, tag="S3run")
            nc.vector.memset(S3run, 0.0)
            M3run = bsc.tile([128, T_CORE], F32, tag="M3run")
            nc.vector.memset(M3run, 0.0)

            # ======== phase A: x prep -> q1/q2 -> feature-major q1T/q2T
            for tt in range(TT):
                tok0 = tt * 128
                x_t = xpool.tile([128, D], F32, tag="x")
                nc.sync.dma_start(out=x_t, in_=x_d[tok0:tok0 + 128, :])
                sink = scr.tile([128, D], F32, tag="scr")
                ssq = parts.tile([128, 1], F32, tag="ssq")
                nc.scalar.activation(out=sink, in_=x_t,
                                     func=mybir.ActivationFunctionType.Square,
                                     accum_out=ssq)
                # r = 1/sqrt(ssq/D + eps)
                nc.scalar.activation(out=r_t[:, tt:tt + 1], in_=ssq,
                                     func=mybir.ActivationFunctionType.Sqrt,
                                     bias=epst, scale=1.0 / D)
                nc.vector.reciprocal(out=r_t[:, tt:tt + 1], in_=r_t[:, tt:tt + 1])

                for (bi, grep, q_dram) in ((0, g1rep, q1_d), (1, g2rep, q2_d)):
                    gx = scr.tile([128, D], F32, tag="scr")
                    nc.vector.tensor_tensor(out=gx, in0=x_t, in1=grep,
                                            op=mybir.AluOpType.mult)
                    M = parts.tile([128, 1], F32, tag="M")
                    nc.vector.tensor_reduce(out=M, in_=gx,
                                            axis=mybir.AxisListType.X,
                                            op=mybir.AluOpType.max,
                                            apply_absolute_value=True)
                    rho = parts.tile([128, 1], F32, tag="rho")
                    tok_scalars(cstack[:, bi * 8 + tt:bi * 8 + tt + 1], rho,
                                M, r_t[:, tt:tt + 1], bi)
                    # q = rint(gx * rho) via magic constant, cast to bf16
                    nc.vector.tensor_scalar(out=gx, in0=gx, scalar1=rho,
                                            scalar2=C_RINT,
                                            op0=mybir.AluOpType.mult,
                                            op1=mybir.AluOpType.add)
                    qt = qb.tile([128, D], BF16, tag="qb")
                    nc.vector.tensor_scalar(out=qt, in0=gx, scalar1=C_RINT,
                                            scalar2=None,
                                            op0=mybir.AluOpType.subtract)
                    nc.scalar.dma_start(out=q_dram[tok0:tok0 + 128, :], in_=qt)
            # feature-major loads via XBAR transpose (d = dc*128 + p layout)
            nc.scalar.dma_start_transpose(q1T, q1_d[:, :])
            nc.scalar.dma_start_transpose(q2T, q2_d[:, :])

            # c1/c2 -> token-broadcast rows (PE transpose + DRAM bounce)
            cps = psB.tile([128, 128], F32, tag="ps", name="cps")
            nc.tensor.transpose(cps[0:16, :], cstack, ident)
            csb = parts.tile([16, 128], F32, tag="csb")
            nc.vector.tensor_copy(out=csb, in_=cps[0:16, :])
            nc.sync.dma_start(
                out=cstat_d.rearrange("one (a b) -> a (one b)", a=16), in_=csb)
            nc.sync.dma_start(
                out=c1rep,
                in_=cstat_d[0:1, 0:T_CORE].to_broadcast([128, T_CORE]))
            nc.sync.dma_start(
                out=c2rep,
                in_=cstat_d[0:1, T_CORE:2 * T_CORE].to_broadcast([128, T_CORE]))

            # ======== phase B: mm1/mm2 feature-major + h + stats
            for hb in range(HB):
                w1b = wpool.tile([128, DC, HBW], BF16, tag="w1b")
                nc.sync.dma_start(out=w1b, in_=w1_r[:, :, hb * HBW:(hb + 1) * HBW])
                w2b = wpool.tile([128, DC, HBW], BF16, tag="w2b")
                nc.sync.dma_start(out=w2b, in_=w2_r[:, :, hb * HBW:(hb + 1) * HBW])
                for hc2 in range(2):
                    ghc = hb * 2 + hc2
                    for th in range(TH):
                        ts = slice(th * 512, (th + 1) * 512)
                        pu = psB.tile([128, 512], F32, tag="ps")
                        for dc in range(DC):
                            nc.tensor.matmul(
                                pu,
                                lhsT=w1b[:, dc, hc2 * 128:(hc2 + 1) * 128],
                                rhs=q1T[:, dc, ts],
                                start=(dc == 0), stop=(dc == DC - 1))
                        pv = psB.tile([128, 512], F32, tag="ps")
                        for dc in range(DC):
                            nc.tensor.matmul(
                                pv,
                                lhsT=w2b[:, dc, hc2 * 128:(hc2 + 1) * 128],
                                rhs=q2T[:, dc, ts],
                                start=(dc == 0), stop=(dc == DC - 1))
                        # h = silu(c1*u) * (c2*v), all [128 H, 512 tok]
                        t1 = hpool.tile([128, 512], F32, tag="h")
                        nc.vector.tensor_tensor(out=t1, in0=pu, in1=c1rep[:, ts],
                                                op=mybir.AluOpType.mult)
                        sg = hpool.tile([128, 512], F32, tag="h")
                        nc.scalar.activation(out=sg, in_=t1,
                                             func=mybir.ActivationFunctionType.Sigmoid)
                        sw = hpool.tile([128, 512], F32, tag="h")
                        nc.vector.tensor_tensor(out=sw, in0=sg, in1=t1,
                                                op=mybir.AluOpType.mult)
                        t2 = hpool.tile([128, 512], F32, tag="h")
                        nc.vector.tensor_tensor(out=t2, in0=pv, in1=c2rep[:, ts],
                                                op=mybir.AluOpType.mult)
                        ht = hpool.tile([128, 512], F32, tag="h")
                        nc.vector.tensor_tensor(out=ht, in0=sw, in1=t2,
                                                op=mybir.AluOpType.mult)
                        # stats accumulate
                        hsq = hpool.tile([128, 512], F32, tag="h")
                        nc.scalar.activation(out=hsq, in_=ht,
                                             func=mybir.ActivationFunctionType.Square)
                        nc.vector.tensor_tensor(out=S3run[:, ts], in0=S3run[:, ts],
                                                in1=hsq, op=mybir.AluOpType.add)
                        gh = hpool.tile([128, 512], F32, tag="h")
                        nc.scalar.activation(out=gh, in_=ht,
                                             func=mybir.ActivationFunctionType.Copy,
                                             scale=g3c_t[:, ghc:ghc + 1])
                        gha = hpool.tile([128, 512], F32, tag="h")
                        nc.scalar.activation(out=gha, in_=ht,
                                             func=mybir.ActivationFunctionType.Abs,
                                             scale=g3c_t[:, ghc:ghc + 1])
                        nc.vector.tensor_tensor(out=M3run[:, ts], in0=M3run[:, ts],
                                                in1=gha, op=mybir.AluOpType.max)
                        nc.scalar.dma_start(
                            out=gh_d[ghc * 128:(ghc + 1) * 128, ts], in_=gh)

            # ======== phase C: transpose stats -> per-token scalars
            for tt in range(TT):
                tok0 = tt * 128
                pts = psB.tile([128, 128], F32, tag="ps", name=f"ptS{tt}")
                nc.tensor.transpose(pts, S3run[:, tok0:tok0 + 128], ident)
                nc.vector.tensor_reduce(out=S3_t[:, tt:tt + 1], in_=pts,
                                        axis=mybir.AxisListType.X,
                                        op=mybir.AluOpType.add)
                ptm = psB.tile([128, 128], F32, tag="ps", name=f"ptM{tt}")
                nc.tensor.transpose(ptm, M3run[:, tok0:tok0 + 128], ident)
                nc.vector.tensor_reduce(out=M3_t[:, tt:tt + 1], in_=ptm,
                                        axis=mybir.AxisListType.X,
                                        op=mybir.AluOpType.max)
                r3 = parts.tile([128, 1], F32, tag="r3")
                nc.scalar.activation(out=r3, in_=S3_t[:, tt:tt + 1],
                                     func=mybir.ActivationFunctionType.Sqrt,
                                     bias=epst, scale=1.0 / H)
                nc.vector.reciprocal(out=r3, in_=r3)
                tok_scalars(c3_t[:, tt:tt + 1], rho3_t[:, tt:tt + 1],
                            M3_t[:, tt:tt + 1], r3, 2)
            # rho3 -> token-broadcast row
            rps = psB.tile([128, 128], F32, tag="ps", name="rps")
            nc.tensor.transpose(rps[0:TT, :], rho3_t, ident)
            rsb = parts.tile([TT, 128], F32, tag="rsb")
            nc.vector.tensor_copy(out=rsb, in_=rps[0:TT, :])
            nc.sync.dma_start(
                out=rstat_d.rearrange("one (a b) -> a (one b)", a=TT), in_=rsb)
            nc.sync.dma_start(out=rho3rep,
                              in_=rstat_d[:, :].to_broadcast([128, T_CORE]))

        # ======== phase D: quantize q3T chunks + mm3 (no transposes)
        with tc.tile_pool(name="q3p", bufs=1) as q3p, \
             tc.tile_pool(name="ghl", bufs=4) as ghlp, \
             tc.tile_pool(name="w3p", bufs=4) as w3p, \
             tc.tile_pool(name="outp", bufs=4) as outp, \
             tc.tile_pool(name="psD", bufs=8, space="PSUM") as psD:

            q3c = []
            for hc in range(HC):
                ghl = ghlp.tile([128, T_CORE], F32, tag="ghl")
                nc.sync.dma_start(out=ghl, in_=gh_d[hc * 128:(hc + 1) * 128, :])
                tq = ghlp.tile([128, T_CORE], F32, tag="tq")
                nc.vector.tensor_tensor(out=tq, in0=ghl, in1=rho3rep,
                                        op=mybir.AluOpType.mult)
                q3 = q3p.tile([128, T_CORE], BF16, tag=f"q3_{hc}")
                nc.vector.tensor_scalar(out=q3, in0=tq, scalar1=C_RINT,
                                        scalar2=C_RINT,
                                        op0=mybir.AluOpType.add,
                                        op1=mybir.AluOpType.subtract)
                q3c.append(q3)

            for dq in range(DQ):
                dcol = dq * 512
                pos = [psD.tile([128, 512], F32, tag="po", name=f"po{dq}_{t}")
                       for t in range(TT)]
                for hc in range(HC):
                    w3b = w3p.tile([128, 512], BF16, tag="w3b")
                    nc.sync.dma_start(
                        out=w3b,
                        in_=w3_d[hc * 128:(hc + 1) * 128, dcol:dcol + 512])
                    for t in range(TT):
                        nc.tensor.matmul(
                            pos[t],
                            lhsT=q3c[hc][:, t * 128:(t + 1) * 128],
                            rhs=w3b,
                            start=(hc == 0), stop=(hc == HC - 1),
                            skip_group_check=True)
                for t in range(TT):
                    ob = outp.tile([128, 512], F32, tag="ob")
                    if t % 2 == 0:
                        nc.scalar.mul(out=ob, in_=pos[t], mul=c3_t[:, t:t + 1])
                    else:
                        nc.vector.tensor_scalar(out=ob, in0=pos[t],
                                                scalar1=c3_t[:, t:t + 1],
                                                scalar2=None,
                                                op0=mybir.AluOpType.mult)
                    nc.scalar.dma_start(
                        out=out_d[t * 128:(t + 1) * 128, dcol:dcol + 512],
                        in_=ob)

    nc.compile()
    return nc


_NC_CACHE = []


def _get_program():
    if not _NC_CACHE:
        _NC_CACHE.append(_build_program())
    return _NC_CACHE[0]


def _ternary_T(w):
    """Host ternarization matching round(tanh(w/(mean|w|+eps))) in value.
    Uses CPU-jax to replicate the reference's fp32 tanh bit-for-bit.
    Returns (transposed ternary bf16 array, arctanh(s) as float32)."""
    w32 = np.asarray(w, dtype=np.float32)
    try:
        import jax
        import jax.numpy as jnp
        cpu = jax.devices("cpu")[0]
        with jax.default_device(cpu):
            s = jnp.mean(jnp.abs(jnp.asarray(w32)))
            t = np.asarray(jnp.round(jnp.tanh(w32 / (s + np.float32(EPS)))))
            a = np.float32(jnp.arctanh(s))
    except Exception:
        s32 = np.float32(np.mean(np.abs(w32), dtype=np.float64))
        denom = np.float32(s32 + np.float32(EPS))
        thresh = np.float32(ATANH_HALF) * denom
        t = np.sign(w32) * (np.abs(w32) > thresh)
        a = np.float32(np.arctanh(np.float64(s32)))
    return np.ascontiguousarray(t.T).astype(ml_dtypes.bfloat16), a


def _make_inputs(x, w1, g1, w2, g2, w3, g3):
    x32 = np.asarray(x, np.float32).reshape(NTOK, D)
    w1q, a1 = _ternary_T(w1)            # [D, H]
    w2q, a2 = _ternary_T(w2)            # [D, H]
    w3q, a3 = _ternary_T(w3)            # [H, D] (w3 is [D, H])
    g1f = np.ascontiguousarray(np.asarray(g1, np.float32).reshape(1, D))
    g2f = np.ascontiguousarray(np.asarray(g2, np.float32).reshape(1, D))
    g3c = np.ascontiguousarray(
        np.asarray(g3, np.float32).reshape(HC, 128).T)   # [128, HC]
    kconst = np.array([[a1 / 127.0, a2 / 127.0, a3 / 127.0]], np.float32)
    in_maps = []
    for c in range(NCORES):
        in_maps.append({
            "x": np.ascontiguousarray(x32[c * T_CORE:(c + 1) * T_CORE]),
            "w1q": w1q, "w2q": w2q, "w3q": w3q,
            "g1": g1f, "g2": g2f, "g3c": g3c,
            "kconst": kconst,
        })
    return in_maps


def kernel(x, w1, g1, w2, g2, w3, g3):
    nc = _get_program()
    in_maps = _make_inputs(x, w1, g1, w2, g2, w3, g3)
    res = run_bass_kernel_spmd(nc, in_maps, list(range(NCORES)))
    out = np.concatenate([res.results[c]["out"] for c in range(NCORES)], axis=0)
    return out.reshape(B, S, D)
